# revision 15
# baseline (speedup 1.0000x reference)
"""MultiHeadAttention forward for Trainium2, 8 NeuronCores.

Problem: B=4, S=2048, D=1024, H=16 heads (head_dim 64), fp32.
  qkv = x @ w_qkv + b_qkv ; q *= hd^-0.5
  attn = softmax(q k^T) ; out = (attn v) @ w_out + b_out

Sharding: 4-way data parallel over batch x 2-way tensor parallel over
heads. Core i handles batch i//2, heads 8*(i%2) .. 8*(i%2)+7. Each core
computes a partial output ([2048, 1024]); the two TP halves of each
batch are summed on the host (each core adds b_out/2 so the sum carries
the full bias).

Dataflow per core (all matmuls in float32r - full PE rate, ~1e-4 rel):
  1. x [S, D] -> PE-transpose -> xT [D, S] (channel-major), streamed in
     512-token chunks.
  2. QKV: qT, kT channel-major [512, S] via lhsT=w block, rhs=xT;
     v token-major [S, 512] via lhsT=xT block, rhs=w_v. The 1/8 query
     scaling is folded into w_q/b_q on the host. v is stored per head
     with an appended ones column ([v_h | 1]) so the attention matmul
     also produces the softmax denominator.
  3. Attention per head pair (A at partitions 0:64, B at 64:128):
     scoresT [tk, tq] = kT_blk.T @ qT ; p = exp(scoresT) (no max
     subtraction needed: |scores| <~ 6); attn_psum [65, tq] accumulates
     v_aug.T @ p over the 16 key tiles - row 64 is the denominator.
     Normalize with DVE reciprocal + DRAM-bounce partition broadcast.
  4. out += attn_outT_blk.T @ w_out_blk accumulated over the 4
     128-channel blocks, + b_out/2, written token-major.
"""

import os

import numpy as np

B, S, D, H, HD = 4, 2048, 1024, 16, 64
NCORES = 8
TPW = 2            # tensor-parallel width over heads
HPC = H // TPW     # heads per core
CH = HPC * HD      # q/k/v channels per core (512)
NPAIR = HPC // 2   # head pairs per core
TT = S // 128      # token tiles
KT = D // 128      # contraction tiles for qkv proj
TC = S // 512      # 512-token chunks for qkv phase
SQC = S // 1024    # 1024-token chunks for attention queries

LAST_RESULTS = None
_CACHED = {}


def _build_nc():
    import concourse.bacc as bacc
    import concourse.mybir as mybir
    import concourse.tile as tile

    F32R = mybir.dt.float32r
    F32 = mybir.dt.float32
    EXP = mybir.ActivationFunctionType.Exp

    nc = bacc.Bacc("TRN2", target_bir_lowering=False)

    x = nc.dram_tensor("x", [S, D], F32R, kind="ExternalInput")
    wqkv = nc.dram_tensor("wqkv", [D, 3 * CH], F32R, kind="ExternalInput")
    bqk = nc.dram_tensor("bqk", [128, 2 * (CH // 128)], F32, kind="ExternalInput")
    bv = nc.dram_tensor("bv", [128, CH], F32, kind="ExternalInput")
    wout = nc.dram_tensor("wout", [CH, D], F32R, kind="ExternalInput")
    bout = nc.dram_tensor("bout", [128, D], F32, kind="ExternalInput")
    ident = nc.dram_tensor("ident", [128, 128], F32R, kind="ExternalInput")
    o = nc.dram_tensor("o", [S, D], F32, kind="ExternalOutput")

    x4 = x.ap().rearrange("(tt p) d -> tt p d", p=128)      # [TT, 128, D]
    w3 = wqkv.ap().rearrange("(kt p) c -> kt p c", p=128)   # [KT, 128, 3CH]
    wo3 = wout.ap().rearrange("(dt p) c -> dt p c", p=128)  # [4, 128, D]
    o4 = o.ap().rearrange("(tt p) d -> tt p d", p=128)

    CT = CH // 128  # 4 channel tiles per q/k

    with tile.TileContext(nc) as tc:
        with (
            tc.tile_pool(name="persist", bufs=1) as pp,
            tc.tile_pool(name="dram", bufs=4, space="DRAM") as dr,
        ):
            qT = pp.tile([128, CT, S], F32R)        # q^T channel-major
            kT = pp.tile([128, CT, S], F32R)
            # v per (tt, pair): [vA | 1 | vB | 1] -> 130 cols
            vaug = pp.tile([128, TT, NPAIR, 130], F32R)
            bqk_sb = pp.tile([128, 2 * CT], F32)
            bv_sb = pp.tile([128, CH], F32)

            nc.sync.dma_start(out=bqk_sb, in_=bqk.ap())
            nc.sync.dma_start(out=bv_sb, in_=bv.ap())
            # ones columns of vaug, via DVE (in0*0 + 1); in0 is just a
            # shape-matched f32 source
            vflat = vaug.rearrange("p tt j c -> p (tt j) c")
            src64 = bv_sb[:, 0:TT * NPAIR].unsqueeze(2)
            nc.vector.tensor_scalar(
                out=vflat[:, :, 64:65], in0=src64, scalar1=0.0, scalar2=1.0,
                op0=mybir.AluOpType.mult, op1=mybir.AluOpType.add)
            nc.vector.tensor_scalar(
                out=vflat[:, :, 129:130], in0=src64, scalar1=0.0, scalar2=1.0,
                op0=mybir.AluOpType.mult, op1=mybir.AluOpType.add)

            # ---------------- Phase A+B: transpose x, QKV projection ----------
            with (
                tc.tile_pool(name="wq", bufs=1) as wq,
                tc.tile_pool(name="xin", bufs=2) as xin,
                tc.tile_pool(name="xtp", bufs=2) as xtp,
                tc.tile_pool(name="psA", bufs=2, space="PSUM") as psA,
                tc.tile_pool(name="psQ", bufs=3, space="PSUM") as psQ,
            ):
                id_sb = wq.tile([128, 128], F32R)
                nc.sync.dma_start(out=id_sb, in_=ident.ap())
                w_sb = wq.tile([128, KT, 3 * CH], F32R)
                for kt in range(KT):
                    nc.sync.dma_start(out=w_sb[:, kt, :], in_=w3[kt])

                for tcn in range(TC):
                    xT = xtp.tile([128, KT, 512], F32R, tag="xT")
                    for ti in range(4):
                        x_in = xin.tile([128, D], F32R, tag="xin")
                        nc.scalar.dma_start(out=x_in, in_=x4[tcn * 4 + ti])
                        for ds in range(KT):
                            pst = psA.tile([128, 128], F32R, tag="tp")
                            nc.tensor.transpose(
                                pst, x_in[:, ds * 128:(ds + 1) * 128], id_sb)
                            nc.vector.tensor_copy(
                                out=xT[:, ds, ti * 128:(ti + 1) * 128], in_=pst)
                    # qT / kT (channel-major)
                    for ct in range(2 * CT):
                        ps = psQ.tile([128, 512], F32, tag="qk")
                        for kt in range(KT):
                            nc.tensor.matmul(
                                ps, w_sb[:, kt, ct * 128:(ct + 1) * 128],
                                xT[:, kt, :],
                                start=(kt == 0), stop=(kt == KT - 1))
                        dst = qT if ct < CT else kT
                        nc.vector.tensor_scalar_add(
                            out=dst[:, ct % CT, tcn * 512:(tcn + 1) * 512],
                            in0=ps, scalar1=bqk_sb[:, ct:ct + 1])
                    # v (token-major, pair-packed with ones cols)
                    for ti in range(4):
                        tt = tcn * 4 + ti
                        psv = psQ.tile([128, CH], F32, tag="v")
                        for kt in range(KT):
                            nc.tensor.matmul(
                                psv, xT[:, kt, ti * 128:(ti + 1) * 128],
                                w_sb[:, kt, 2 * CH:3 * CH],
                                start=(kt == 0), stop=(kt == KT - 1))
                        psv4 = psv.rearrange("p (j two c) -> p j two c", two=2, c=64)
                        bv4 = bv_sb.rearrange("p (j two c) -> p j two c", two=2, c=64)
                        nc.vector.tensor_add(
                            out=vaug[:, tt, :, 0:64], in0=psv4[:, :, 0, :],
                            in1=bv4[:, :, 0, :])
                        nc.vector.tensor_add(
                            out=vaug[:, tt, :, 65:129], in0=psv4[:, :, 1, :],
                            in1=bv4[:, :, 1, :])

            # ---------------- Phase C: attention ------------------------------
            with tc.tile_pool(name="aout", bufs=1) as ao:
                attn_outT = ao.tile([128, CT, S], F32R)
                wo_sb = ao.tile([128, CT, D], F32R)
                bout_sb = ao.tile([128, D], F32)
                nc.sync.dma_start(out=bout_sb, in_=bout.ap())
                for dt in range(CT):
                    nc.sync.dma_start(out=wo_sb[:, dt, :], in_=wo3[dt])
                with (
                    tc.tile_pool(name="pT", bufs=3) as pTp,
                    tc.tile_pool(name="rcp", bufs=2) as rcp,
                    tc.tile_pool(name="rb", bufs=2) as rbp,
                    tc.tile_pool(name="psS", bufs=2, space="PSUM") as psS,
                    tc.tile_pool(name="psAt", bufs=4, space="PSUM") as psAt,
                ):
                    for j in range(NPAIR):
                        for qc in range(SQC):
                            q0 = qc * 1024
                            aA = [psAt.tile([65, 512], F32, tag="at", name=f"aA{j}_{qc}_{h2}") for h2 in range(2)]
                            aB = [psAt.tile([65, 512], F32, tag="at", name=f"aB{j}_{qc}_{h2}") for h2 in range(2)]
                            for kt in range(TT):
                                k0 = kt * 128
                                # One psum tile holds BOTH heads' scores for a
                                # 512-query chunk: cols 0:512 head A (rows
                                # 0:64 of the array), cols 512:1024 head B
                                # (rows 64:128). The two matmuls have no
                                # mutual deps and disjoint row groups, so the
                                # PE runs them concurrently; the exp consumes
                                # both at once.
                                for h2 in range(2):
                                    qq = q0 + h2 * 512
                                    s = psS.tile([128, 1024], F32, tag="s",
                                                 name=f"s{j}_{qc}_{kt}_{h2}")
                                    nc.tensor.matmul(
                                        s[:, 0:512],
                                        kT[0:64, j, k0:k0 + 128],
                                        qT[0:64, j, qq:qq + 512],
                                        start=True, stop=True, tile_position=(0, 0))
                                    nc.tensor.matmul(
                                        s[:, 512:1024],
                                        kT[64:128, j, k0:k0 + 128],
                                        qT[64:128, j, qq:qq + 512],
                                        start=True, stop=True, tile_position=(64, 0))
                                    pT = pTp.tile([128, 1024], F32R, tag="p",
                                                  name=f"pT{j}_{qc}_{kt}_{h2}")
                                    nc.scalar.activation(out=pT, in_=s, func=EXP)
                                    nc.tensor.matmul(
                                        aA[h2], vaug[:, kt, j, 0:65],
                                        pT[:, 0:512],
                                        start=(kt == 0), stop=(kt == TT - 1))
                                    nc.tensor.matmul(
                                        aB[h2], vaug[:, kt, j, 65:130],
                                        pT[:, 512:1024],
                                        start=(kt == 0), stop=(kt == TT - 1))
                            # evacuate + normalize.
                            # Denominators (psum row 64 of the 4 attn psums)
                            # -> DRAM -> one [128, 16] tile so the reciprocal
                            # runs 128 partitions wide (a [1, 512] DVE
                            # reciprocal costs 3.2us) -> DRAM -> partition-
                            # broadcast back as [64, 512] tiles.
                            dsb = rcp.tile([128, 512], F32, tag="ds")
                            nc.vector.tensor_copy(out=dsb[0:1, :], in_=aA[0][64:65, :])
                            nc.vector.tensor_copy(out=dsb[32:33, :], in_=aA[1][64:65, :])
                            nc.vector.tensor_copy(out=dsb[64:65, :], in_=aB[0][64:65, :])
                            nc.vector.tensor_copy(out=dsb[96:97, :], in_=aB[1][64:65, :])
                            rsb = rcp.tile([128, 512], F32, tag="rs")
                            nc.vector.reciprocal(out=rsb[0:97, :], in_=dsb[0:97, :])
                            dden = dr.tile([4, 512], F32, tag="d")
                            nc.sync.dma_start(out=dden, in_=rsb[::32, :])
                            # evacuate psums unnormalized (frees the attn psum
                            # banks without waiting on the reciprocal chain)
                            for h2 in range(2):
                                c0 = q0 + h2 * 512
                                nc.vector.tensor_copy(
                                    out=attn_outT[0:64, j, c0:c0 + 512],
                                    in_=aA[h2][0:64, :])
                                nc.vector.tensor_copy(
                                    out=attn_outT[64:128, j, c0:c0 + 512],
                                    in_=aB[h2][0:64, :])
                            rb4 = rbp.tile([128, 2, 512], F32, tag="rb")
                            for i in range(4):
                                p0 = 0 if i < 2 else 64
                                nc.gpsimd.dma_start(
                                    out=rb4[p0:p0 + 64, i % 2, :],
                                    in_=dden[i:i + 1, :].to_broadcast([64, 512]))
                            for h2 in range(2):
                                c0 = q0 + h2 * 512
                                nc.vector.tensor_mul(
                                    out=attn_outT[0:64, j, c0:c0 + 512],
                                    in0=attn_outT[0:64, j, c0:c0 + 512],
                                    in1=rb4[0:64, h2, :])
                                nc.vector.tensor_mul(
                                    out=attn_outT[64:128, j, c0:c0 + 512],
                                    in0=attn_outT[64:128, j, c0:c0 + 512],
                                    in1=rb4[64:128, h2, :])

                # ---------------- Phase D: output projection ------------------
                with (
                    tc.tile_pool(name="osb", bufs=3) as osb,
                    tc.tile_pool(name="psD", bufs=4, space="PSUM") as psD,
                ):
                    for tt in range(TT):
                        pd = [psD.tile([128, 512], F32, tag="pd", name=f"pd{tt}_{h2}") for h2 in range(2)]
                        for dt in range(CT):
                            for h2 in range(2):
                                nc.tensor.matmul(
                                    pd[h2],
                                    attn_outT[:, dt, tt * 128:(tt + 1) * 128],
                                    wo_sb[:, dt, h2 * 512:(h2 + 1) * 512],
                                    start=(dt == 0), stop=(dt == CT - 1))
                        ot = osb.tile([128, D], F32, tag="o")
                        for h2 in range(2):
                            nc.vector.tensor_add(
                                out=ot[:, h2 * 512:(h2 + 1) * 512], in0=pd[h2],
                                in1=bout_sb[:, h2 * 512:(h2 + 1) * 512])
                        nc.sync.dma_start(out=o4[tt], in_=ot)

    nc.finalize()
    return nc


def _get_nc():
    if "nc" not in _CACHED:
        _CACHED["nc"] = _build_nc()
    return _CACHED["nc"]


def _core_inputs(x, w_qkv, b_qkv, w_out, b_out):
    """Build the 8 per-core input dicts (host-side sharding)."""
    x = np.asarray(x, dtype=np.float32)
    w_qkv = np.asarray(w_qkv, dtype=np.float32)
    b_qkv = np.asarray(b_qkv, dtype=np.float32)
    w_out = np.asarray(w_out, dtype=np.float32)
    b_out = np.asarray(b_out, dtype=np.float32)

    scale = np.float32(HD ** -0.5)
    ident = np.eye(128, dtype=np.float32)
    bout_half = (b_out * np.float32(1.0 / TPW)).astype(np.float32)

    in_maps = []
    for core in range(NCORES):
        b, r = divmod(core, TPW)
        cs = slice(CH * r, CH * (r + 1))
        wq = w_qkv[:, 0:D][:, cs] * scale
        wk = w_qkv[:, D:2 * D][:, cs]
        wv = w_qkv[:, 2 * D:3 * D][:, cs]
        w_pack = np.ascontiguousarray(
            np.concatenate([wq, wk, wv], axis=1), dtype=np.float32)
        bqk_pack = np.concatenate(
            [b_qkv[0:D][cs] * scale, b_qkv[D:2 * D][cs]])
        # [2CH] -> [128, 2CT] with bqk[ct*128+p] at [p, ct]
        bqk_t = np.ascontiguousarray(
            bqk_pack.reshape(-1, 128).T, dtype=np.float32)
        bv_r = np.ascontiguousarray(
            np.broadcast_to(b_qkv[2 * D:3 * D][cs], (128, CH)), dtype=np.float32)
        bout_r = np.ascontiguousarray(
            np.broadcast_to(bout_half, (128, D)), dtype=np.float32)
        wout_r = np.ascontiguousarray(w_out[cs, :], dtype=np.float32)
        in_maps.append({
            "x": np.ascontiguousarray(x[b]),
            "wqkv": w_pack,
            "bqk": bqk_t,
            "bv": bv_r,
            "wout": wout_r,
            "bout": bout_r,
            "ident": ident,
        })
    return in_maps


def _ensure_ntff_hook():
    """Register the axon NTFF profile hook (missing antenv.axon_hooks stub)."""
    import sys
    import types

    if "antenv.axon_hooks" in sys.modules:
        return
    try:
        from trn_agent_boot.trn_boot import _ntff_profile_via_ctypes

        hook = _ntff_profile_via_ctypes("/opt/axon/libaxon_pjrt.so")
        if hook is None:
            return
        mod = types.ModuleType("antenv.axon_hooks")
        mod.get_axon_ntff_profile_hook = lambda: hook
        mod.set_axon_ntff_profile_hook = lambda h: None
        sys.modules["antenv.axon_hooks"] = mod
    except Exception:
        pass


def kernel(x, w_qkv, b_qkv, w_out, b_out):
    global LAST_RESULTS
    from concourse.bass_utils import run_bass_kernel_spmd

    nc = _get_nc()
    in_maps = _core_inputs(x, w_qkv, b_qkv, w_out, b_out)
    trace = bool(os.environ.get("BASS_TRACE"))
    if trace:
        _ensure_ntff_hook()
    res = run_bass_kernel_spmd(
        nc, in_maps, core_ids=list(range(NCORES)), trace=trace)
    LAST_RESULTS = res
    out = np.empty((B, S, D), dtype=np.float32)
    for b in range(B):
        out[b] = res.results[TPW * b]["o"]
        for r in range(1, TPW):
            out[b] += res.results[TPW * b + r]["o"]
    return out


# revision 17
# speedup vs baseline: 1.2258x; 1.2258x over previous
"""MultiHeadAttention forward for Trainium2, 8 NeuronCores.

Problem: B=4, S=2048, D=1024, H=16 heads (head_dim 64), fp32.
  qkv = x @ w_qkv + b_qkv ; q *= hd^-0.5
  attn = softmax(q k^T) ; out = (attn v) @ w_out + b_out

Sharding: 4-way data parallel over batch x 2-way tensor parallel over
heads. Core i handles batch i//2, heads 8*(i%2) .. 8*(i%2)+7. Each core
computes a partial output ([2048, 1024]); the two TP halves of each
batch are summed on the host (each core adds b_out/2 so the sum carries
the full bias).

Dataflow per core (all matmuls in float32r - full PE rate, ~1e-4 rel):
  1. x [S, D] -> PE-transpose -> xT [D, S] (channel-major), streamed in
     512-token chunks.
  2. QKV: qT, kT channel-major [512, S] via lhsT=w block, rhs=xT;
     v token-major [S, 512] via lhsT=xT block, rhs=w_v. The 1/8 query
     scaling is folded into w_q/b_q on the host. v is stored per head
     with an appended ones column ([v_h | 1]) so the attention matmul
     also produces the softmax denominator.
  3. Attention per head pair (A at partitions 0:64, B at 64:128):
     scoresT [tk, tq] = kT_blk.T @ qT ; p = exp(scoresT) (no max
     subtraction needed: |scores| <~ 6); attn_psum [65, tq] accumulates
     v_aug.T @ p over the 16 key tiles - row 64 is the denominator.
     Normalize with DVE reciprocal + DRAM-bounce partition broadcast.
  4. out += attn_outT_blk.T @ w_out_blk accumulated over the 4
     128-channel blocks, + b_out/2, written token-major.
"""

import os

import numpy as np

B, S, D, H, HD = 4, 2048, 1024, 16, 64
NCORES = 8
TPW = 2            # tensor-parallel width over heads
HPC = H // TPW     # heads per core
CH = HPC * HD      # q/k/v channels per core (512)
NPAIR = HPC // 2   # head pairs per core
TT = S // 128      # token tiles
KT = D // 128      # contraction tiles for qkv proj
TC = S // 512      # 512-token chunks for qkv phase
SQC = S // 1024    # 1024-token chunks for attention queries

LAST_RESULTS = None
_CACHED = {}


def _build_nc():
    import concourse.bacc as bacc
    import concourse.mybir as mybir
    import concourse.tile as tile

    F32R = mybir.dt.float32r
    F32 = mybir.dt.float32
    EXP = mybir.ActivationFunctionType.Exp

    nc = bacc.Bacc("TRN2", target_bir_lowering=False)

    x = nc.dram_tensor("x", [S, D], F32R, kind="ExternalInput")
    wqkv = nc.dram_tensor("wqkv", [D, 3 * CH], F32R, kind="ExternalInput")
    bqk = nc.dram_tensor("bqk", [128, 2 * (CH // 128)], F32, kind="ExternalInput")
    bv = nc.dram_tensor("bv", [128, CH], F32, kind="ExternalInput")
    wout = nc.dram_tensor("wout", [CH, D], F32R, kind="ExternalInput")
    bout = nc.dram_tensor("bout", [128, D], F32, kind="ExternalInput")
    ident = nc.dram_tensor("ident", [128, 128], F32R, kind="ExternalInput")
    o = nc.dram_tensor("o", [S, D], F32, kind="ExternalOutput")

    x4 = x.ap().rearrange("(tt p) d -> tt p d", p=128)      # [TT, 128, D]
    w3 = wqkv.ap().rearrange("(kt p) c -> kt p c", p=128)   # [KT, 128, 3CH]
    wo3 = wout.ap().rearrange("(dt p) c -> dt p c", p=128)  # [4, 128, D]
    o4 = o.ap().rearrange("(tt p) d -> tt p d", p=128)

    CT = CH // 128  # 4 channel tiles per q/k

    with tile.TileContext(nc) as tc:
        with (
            tc.tile_pool(name="persist", bufs=1) as pp,
            tc.tile_pool(name="dram", bufs=4, space="DRAM") as dr,
        ):
            qT = pp.tile([128, CT, S], F32R)        # q^T channel-major
            kT = pp.tile([128, CT, S], F32R)
            # v per (tt, pair): [vA | 1 | vB | 1] -> 130 cols
            vaug = pp.tile([128, TT, NPAIR, 130], F32R)
            bqk_sb = pp.tile([128, 2 * CT], F32)
            bv_sb = pp.tile([128, CH], F32)

            nc.sync.dma_start(out=bqk_sb, in_=bqk.ap())
            nc.sync.dma_start(out=bv_sb, in_=bv.ap())
            # ones columns of vaug, via DVE (in0*0 + 1); in0 is just a
            # shape-matched f32 source
            vflat = vaug.rearrange("p tt j c -> p (tt j) c")
            src64 = bv_sb[:, 0:TT * NPAIR].unsqueeze(2)
            nc.vector.tensor_scalar(
                out=vflat[:, :, 64:65], in0=src64, scalar1=0.0, scalar2=1.0,
                op0=mybir.AluOpType.mult, op1=mybir.AluOpType.add)
            nc.vector.tensor_scalar(
                out=vflat[:, :, 129:130], in0=src64, scalar1=0.0, scalar2=1.0,
                op0=mybir.AluOpType.mult, op1=mybir.AluOpType.add)

            # ---------------- Phase A+B: transpose x, QKV projection ----------
            with (
                tc.tile_pool(name="wq", bufs=1) as wq,
                tc.tile_pool(name="xin", bufs=2) as xin,
                tc.tile_pool(name="xtp", bufs=2) as xtp,
                tc.tile_pool(name="psA", bufs=2, space="PSUM") as psA,
                tc.tile_pool(name="psQ", bufs=3, space="PSUM") as psQ,
            ):
                id_sb = wq.tile([128, 128], F32R)
                nc.sync.dma_start(out=id_sb, in_=ident.ap())
                w_sb = wq.tile([128, KT, 3 * CH], F32R)
                for kt in range(KT):
                    nc.sync.dma_start(out=w_sb[:, kt, :], in_=w3[kt])

                for tcn in range(TC):
                    xT = xtp.tile([128, KT, 512], F32R, tag="xT")
                    for ti in range(4):
                        x_in = xin.tile([128, D], F32R, tag="xin")
                        nc.scalar.dma_start(out=x_in, in_=x4[tcn * 4 + ti])
                        for ds in range(KT):
                            pst = psA.tile([128, 128], F32R, tag="tp")
                            nc.tensor.transpose(
                                pst, x_in[:, ds * 128:(ds + 1) * 128], id_sb)
                            nc.vector.tensor_copy(
                                out=xT[:, ds, ti * 128:(ti + 1) * 128], in_=pst)
                    # qT / kT (channel-major)
                    for ct in range(2 * CT):
                        ps = psQ.tile([128, 512], F32, tag="qk")
                        for kt in range(KT):
                            nc.tensor.matmul(
                                ps, w_sb[:, kt, ct * 128:(ct + 1) * 128],
                                xT[:, kt, :],
                                start=(kt == 0), stop=(kt == KT - 1))
                        dst = qT if ct < CT else kT
                        nc.vector.tensor_scalar_add(
                            out=dst[:, ct % CT, tcn * 512:(tcn + 1) * 512],
                            in0=ps, scalar1=bqk_sb[:, ct:ct + 1])
                    # v (token-major, pair-packed with ones cols)
                    for ti in range(4):
                        tt = tcn * 4 + ti
                        psv = psQ.tile([128, CH], F32, tag="v")
                        for kt in range(KT):
                            nc.tensor.matmul(
                                psv, xT[:, kt, ti * 128:(ti + 1) * 128],
                                w_sb[:, kt, 2 * CH:3 * CH],
                                start=(kt == 0), stop=(kt == KT - 1))
                        psv4 = psv.rearrange("p (j two c) -> p j two c", two=2, c=64)
                        bv4 = bv_sb.rearrange("p (j two c) -> p j two c", two=2, c=64)
                        nc.vector.tensor_add(
                            out=vaug[:, tt, :, 0:64], in0=psv4[:, :, 0, :],
                            in1=bv4[:, :, 0, :])
                        nc.vector.tensor_add(
                            out=vaug[:, tt, :, 65:129], in0=psv4[:, :, 1, :],
                            in1=bv4[:, :, 1, :])

            # ---------------- Phase C: attention ------------------------------
            with tc.tile_pool(name="aout", bufs=1) as ao:
                attn_outT = ao.tile([128, CT, S], F32R)
                wo_sb = ao.tile([128, CT, D], F32R)
                bout_sb = ao.tile([128, D], F32)
                nc.sync.dma_start(out=bout_sb, in_=bout.ap())
                for dt in range(CT):
                    nc.sync.dma_start(out=wo_sb[:, dt, :], in_=wo3[dt])
                with (
                    tc.tile_pool(name="pT", bufs=3) as pTp,
                    tc.tile_pool(name="rcp", bufs=2) as rcp,
                    tc.tile_pool(name="rb", bufs=2) as rbp,
                    tc.tile_pool(name="psS", bufs=2, space="PSUM") as psS,
                    tc.tile_pool(name="psAt", bufs=4, space="PSUM") as psAt,
                ):
                    for j in range(NPAIR):
                        for qc in range(SQC):
                            q0 = qc * 1024
                            aA = [psAt.tile([65, 512], F32, tag="at", name=f"aA{j}_{qc}_{h2}") for h2 in range(2)]
                            aB = [psAt.tile([65, 512], F32, tag="at", name=f"aB{j}_{qc}_{h2}") for h2 in range(2)]
                            for kt in range(TT):
                                k0 = kt * 128
                                # One psum tile holds BOTH heads' scores for a
                                # 512-query chunk: cols 0:512 head A (rows
                                # 0:64 of the array), cols 512:1024 head B
                                # (rows 64:128). The two matmuls have no
                                # mutual deps and disjoint row groups, so the
                                # PE runs them concurrently; the exp consumes
                                # both at once.
                                for h2 in range(2):
                                    qq = q0 + h2 * 512
                                    s = psS.tile([128, 1024], F32, tag="s",
                                                 name=f"s{j}_{qc}_{kt}_{h2}")
                                    nc.tensor.matmul(
                                        s[:, 0:512],
                                        kT[0:64, j, k0:k0 + 128],
                                        qT[0:64, j, qq:qq + 512],
                                        start=True, stop=True, tile_position=(0, 0))
                                    nc.tensor.matmul(
                                        s[:, 512:1024],
                                        kT[64:128, j, k0:k0 + 128],
                                        qT[64:128, j, qq:qq + 512],
                                        start=True, stop=True, tile_position=(64, 0))
                                    pT = pTp.tile([128, 1024], F32R, tag="p",
                                                  name=f"pT{j}_{qc}_{kt}_{h2}")
                                    nc.scalar.activation(out=pT, in_=s, func=EXP)
                                    nc.tensor.matmul(
                                        aA[h2], vaug[:, kt, j, 0:65],
                                        pT[:, 0:512],
                                        start=(kt == 0), stop=(kt == TT - 1))
                                    nc.tensor.matmul(
                                        aB[h2], vaug[:, kt, j, 65:130],
                                        pT[:, 512:1024],
                                        start=(kt == 0), stop=(kt == TT - 1))
                            # evacuate + normalize.
                            # Denominators (psum row 64 of the 4 attn psums)
                            # -> DRAM -> one [128, 16] tile so the reciprocal
                            # runs 128 partitions wide (a [1, 512] DVE
                            # reciprocal costs 3.2us) -> DRAM -> partition-
                            # broadcast back as [64, 512] tiles.
                            dsb = rcp.tile([128, 512], F32, tag="ds")
                            nc.vector.tensor_copy(out=dsb[0:1, :], in_=aA[0][64:65, :])
                            nc.vector.tensor_copy(out=dsb[32:33, :], in_=aA[1][64:65, :])
                            nc.vector.tensor_copy(out=dsb[64:65, :], in_=aB[0][64:65, :])
                            nc.vector.tensor_copy(out=dsb[96:97, :], in_=aB[1][64:65, :])
                            dden = dr.tile([4, 512], F32, tag="d")
                            nc.sync.dma_start(out=dden, in_=dsb[::32, :])
                            dgat = rcp.tile([128, 16], F32, tag="g")
                            nc.sync.dma_start(
                                out=dgat,
                                in_=dden.rearrange("f (i p) -> p f i", p=128))
                            rgat = rcp.tile([128, 16], F32, tag="g")
                            nc.vector.reciprocal(out=rgat, in_=dgat)
                            drec = dr.tile([4, 512], F32, tag="d")
                            nc.sync.dma_start(
                                out=drec.rearrange("f (i p) -> p f i", p=128),
                                in_=rgat)
                            # evacuate psums unnormalized (frees the attn psum
                            # banks without waiting on the reciprocal chain)
                            for h2 in range(2):
                                c0 = q0 + h2 * 512
                                nc.vector.tensor_copy(
                                    out=attn_outT[0:64, j, c0:c0 + 512],
                                    in_=aA[h2][0:64, :])
                                nc.vector.tensor_copy(
                                    out=attn_outT[64:128, j, c0:c0 + 512],
                                    in_=aB[h2][0:64, :])
                            rb4 = rbp.tile([128, 2, 512], F32, tag="rb")
                            for i in range(4):
                                p0 = 0 if i < 2 else 64
                                nc.gpsimd.dma_start(
                                    out=rb4[p0:p0 + 64, i % 2, :],
                                    in_=drec[i:i + 1, :].to_broadcast([64, 512]))
                            for h2 in range(2):
                                c0 = q0 + h2 * 512
                                nc.vector.tensor_mul(
                                    out=attn_outT[0:64, j, c0:c0 + 512],
                                    in0=attn_outT[0:64, j, c0:c0 + 512],
                                    in1=rb4[0:64, h2, :])
                                nc.vector.tensor_mul(
                                    out=attn_outT[64:128, j, c0:c0 + 512],
                                    in0=attn_outT[64:128, j, c0:c0 + 512],
                                    in1=rb4[64:128, h2, :])

                # ---------------- Phase D: output projection ------------------
                with (
                    tc.tile_pool(name="osb", bufs=3) as osb,
                    tc.tile_pool(name="psD", bufs=4, space="PSUM") as psD,
                ):
                    for tt in range(TT):
                        pd = [psD.tile([128, 512], F32, tag="pd", name=f"pd{tt}_{h2}") for h2 in range(2)]
                        for dt in range(CT):
                            for h2 in range(2):
                                nc.tensor.matmul(
                                    pd[h2],
                                    attn_outT[:, dt, tt * 128:(tt + 1) * 128],
                                    wo_sb[:, dt, h2 * 512:(h2 + 1) * 512],
                                    start=(dt == 0), stop=(dt == CT - 1))
                        ot = osb.tile([128, D], F32, tag="o")
                        for h2 in range(2):
                            nc.vector.tensor_add(
                                out=ot[:, h2 * 512:(h2 + 1) * 512], in0=pd[h2],
                                in1=bout_sb[:, h2 * 512:(h2 + 1) * 512])
                        nc.sync.dma_start(out=o4[tt], in_=ot)

    nc.finalize()
    return nc


def _get_nc():
    if "nc" not in _CACHED:
        _CACHED["nc"] = _build_nc()
    return _CACHED["nc"]


def _core_inputs(x, w_qkv, b_qkv, w_out, b_out):
    """Build the 8 per-core input dicts (host-side sharding)."""
    x = np.asarray(x, dtype=np.float32)
    w_qkv = np.asarray(w_qkv, dtype=np.float32)
    b_qkv = np.asarray(b_qkv, dtype=np.float32)
    w_out = np.asarray(w_out, dtype=np.float32)
    b_out = np.asarray(b_out, dtype=np.float32)

    scale = np.float32(HD ** -0.5)
    ident = np.eye(128, dtype=np.float32)
    bout_half = (b_out * np.float32(1.0 / TPW)).astype(np.float32)

    in_maps = []
    for core in range(NCORES):
        b, r = divmod(core, TPW)
        cs = slice(CH * r, CH * (r + 1))
        wq = w_qkv[:, 0:D][:, cs] * scale
        wk = w_qkv[:, D:2 * D][:, cs]
        wv = w_qkv[:, 2 * D:3 * D][:, cs]
        w_pack = np.ascontiguousarray(
            np.concatenate([wq, wk, wv], axis=1), dtype=np.float32)
        bqk_pack = np.concatenate(
            [b_qkv[0:D][cs] * scale, b_qkv[D:2 * D][cs]])
        # [2CH] -> [128, 2CT] with bqk[ct*128+p] at [p, ct]
        bqk_t = np.ascontiguousarray(
            bqk_pack.reshape(-1, 128).T, dtype=np.float32)
        bv_r = np.ascontiguousarray(
            np.broadcast_to(b_qkv[2 * D:3 * D][cs], (128, CH)), dtype=np.float32)
        bout_r = np.ascontiguousarray(
            np.broadcast_to(bout_half, (128, D)), dtype=np.float32)
        wout_r = np.ascontiguousarray(w_out[cs, :], dtype=np.float32)
        in_maps.append({
            "x": np.ascontiguousarray(x[b]),
            "wqkv": w_pack,
            "bqk": bqk_t,
            "bv": bv_r,
            "wout": wout_r,
            "bout": bout_r,
            "ident": ident,
        })
    return in_maps


def _ensure_ntff_hook():
    """Register the axon NTFF profile hook (missing antenv.axon_hooks stub)."""
    import sys
    import types

    if "antenv.axon_hooks" in sys.modules:
        return
    try:
        from trn_agent_boot.trn_boot import _ntff_profile_via_ctypes

        hook = _ntff_profile_via_ctypes("/opt/axon/libaxon_pjrt.so")
        if hook is None:
            return
        mod = types.ModuleType("antenv.axon_hooks")
        mod.get_axon_ntff_profile_hook = lambda: hook
        mod.set_axon_ntff_profile_hook = lambda h: None
        sys.modules["antenv.axon_hooks"] = mod
    except Exception:
        pass


def kernel(x, w_qkv, b_qkv, w_out, b_out):
    global LAST_RESULTS
    from concourse.bass_utils import run_bass_kernel_spmd

    nc = _get_nc()
    in_maps = _core_inputs(x, w_qkv, b_qkv, w_out, b_out)
    trace = bool(os.environ.get("BASS_TRACE"))
    if trace:
        _ensure_ntff_hook()
    res = run_bass_kernel_spmd(
        nc, in_maps, core_ids=list(range(NCORES)), trace=trace)
    LAST_RESULTS = res
    out = np.empty((B, S, D), dtype=np.float32)
    for b in range(B):
        out[b] = res.results[TPW * b]["o"]
        for r in range(1, TPW):
            out[b] += res.results[TPW * b + r]["o"]
    return out


# revision 18
# speedup vs baseline: 1.2534x; 1.0225x over previous
"""MultiHeadAttention forward for Trainium2, 8 NeuronCores.

Problem: B=4, S=2048, D=1024, H=16 heads (head_dim 64), fp32.
  qkv = x @ w_qkv + b_qkv ; q *= hd^-0.5
  attn = softmax(q k^T) ; out = (attn v) @ w_out + b_out

Sharding: 4-way data parallel over batch x 2-way tensor parallel over
heads. Core i handles batch i//2, heads 8*(i%2) .. 8*(i%2)+7. Each core
computes a partial output ([2048, 1024]); the two TP halves of each
batch are summed on the host (each core adds b_out/2 so the sum carries
the full bias).

Dataflow per core (all matmuls in float32r - full PE rate, ~1e-4 rel):
  1. x [S, D] -> PE-transpose -> xT [D, S] (channel-major), streamed in
     512-token chunks.
  2. QKV: qT, kT channel-major [512, S] via lhsT=w block, rhs=xT;
     v token-major [S, 512] via lhsT=xT block, rhs=w_v. The 1/8 query
     scaling is folded into w_q/b_q on the host. v is stored per head
     with an appended ones column ([v_h | 1]) so the attention matmul
     also produces the softmax denominator.
  3. Attention per head pair (A at partitions 0:64, B at 64:128):
     scoresT [tk, tq] = kT_blk.T @ qT ; p = exp(scoresT) (no max
     subtraction needed: |scores| <~ 6); attn_psum [65, tq] accumulates
     v_aug.T @ p over the 16 key tiles - row 64 is the denominator.
     Normalize with DVE reciprocal + DRAM-bounce partition broadcast.
  4. out += attn_outT_blk.T @ w_out_blk accumulated over the 4
     128-channel blocks, + b_out/2, written token-major.
"""

import os

import numpy as np

B, S, D, H, HD = 4, 2048, 1024, 16, 64
NCORES = 8
TPW = 2            # tensor-parallel width over heads
HPC = H // TPW     # heads per core
CH = HPC * HD      # q/k/v channels per core (512)
NPAIR = HPC // 2   # head pairs per core
TT = S // 128      # token tiles
KT = D // 128      # contraction tiles for qkv proj
TC = S // 512      # 512-token chunks for qkv phase
SQC = S // 1024    # 1024-token chunks for attention queries

LAST_RESULTS = None
_CACHED = {}


def _build_nc():
    import concourse.bacc as bacc
    import concourse.mybir as mybir
    import concourse.tile as tile

    F32R = mybir.dt.float32r
    F32 = mybir.dt.float32
    EXP = mybir.ActivationFunctionType.Exp

    nc = bacc.Bacc("TRN2", target_bir_lowering=False)

    x = nc.dram_tensor("x", [S, D], F32R, kind="ExternalInput")
    wqkv = nc.dram_tensor("wqkv", [D, 3 * CH], F32R, kind="ExternalInput")
    bqk = nc.dram_tensor("bqk", [128, 2 * (CH // 128)], F32, kind="ExternalInput")
    bv = nc.dram_tensor("bv", [128, CH], F32, kind="ExternalInput")
    wout = nc.dram_tensor("wout", [CH, D], F32R, kind="ExternalInput")
    bout = nc.dram_tensor("bout", [128, D], F32, kind="ExternalInput")
    ident = nc.dram_tensor("ident", [128, 128], F32R, kind="ExternalInput")
    o = nc.dram_tensor("o", [S, D], F32, kind="ExternalOutput")

    x4 = x.ap().rearrange("(tt p) d -> tt p d", p=128)      # [TT, 128, D]
    w3 = wqkv.ap().rearrange("(kt p) c -> kt p c", p=128)   # [KT, 128, 3CH]
    wo3 = wout.ap().rearrange("(dt p) c -> dt p c", p=128)  # [4, 128, D]
    o4 = o.ap().rearrange("(tt p) d -> tt p d", p=128)

    CT = CH // 128  # 4 channel tiles per q/k

    with tile.TileContext(nc) as tc:
        with (
            tc.tile_pool(name="persist", bufs=1) as pp,
            tc.tile_pool(name="dram", bufs=4, space="DRAM") as dr,
        ):
            qT = pp.tile([128, CT, S], F32R)        # q^T channel-major
            kT = pp.tile([128, CT, S], F32R)
            # v per (tt, pair): [vA | 1 | vB | 1] -> 130 cols
            vaug = pp.tile([128, TT, NPAIR, 130], F32R)
            bqk_sb = pp.tile([128, 2 * CT], F32)
            bv_sb = pp.tile([128, CH], F32)

            nc.sync.dma_start(out=bqk_sb, in_=bqk.ap())
            nc.sync.dma_start(out=bv_sb, in_=bv.ap())
            # ones columns of vaug, via DVE (in0*0 + 1); in0 is just a
            # shape-matched f32 source
            vflat = vaug.rearrange("p tt j c -> p (tt j) c")
            src64 = bv_sb[:, 0:TT * NPAIR].unsqueeze(2)
            nc.vector.tensor_scalar(
                out=vflat[:, :, 64:65], in0=src64, scalar1=0.0, scalar2=1.0,
                op0=mybir.AluOpType.mult, op1=mybir.AluOpType.add)
            nc.vector.tensor_scalar(
                out=vflat[:, :, 129:130], in0=src64, scalar1=0.0, scalar2=1.0,
                op0=mybir.AluOpType.mult, op1=mybir.AluOpType.add)

            # ---------------- Phase A+B: transpose x, QKV projection ----------
            with (
                tc.tile_pool(name="wq", bufs=1) as wq,
                tc.tile_pool(name="xin", bufs=2) as xin,
                tc.tile_pool(name="xtp", bufs=2) as xtp,
                tc.tile_pool(name="psA", bufs=2, space="PSUM") as psA,
                tc.tile_pool(name="psQ", bufs=3, space="PSUM") as psQ,
            ):
                id_sb = wq.tile([128, 128], F32R)
                nc.sync.dma_start(out=id_sb, in_=ident.ap())
                w_sb = wq.tile([128, KT, 3 * CH], F32R)
                for kt in range(KT):
                    for h3 in range(2):
                        c0 = h3 * 3 * CH // 2
                        c1 = (h3 + 1) * 3 * CH // 2
                        nc.sync.dma_start(
                            out=w_sb[:, kt, c0:c1], in_=w3[kt][:, c0:c1])

                for tcn in range(TC):
                    xT = xtp.tile([128, KT, 512], F32R, tag="xT")
                    for ti in range(4):
                        x_in = xin.tile([128, D], F32R, tag="xin")
                        for q4 in range(4):
                            nc.scalar.dma_start(
                                out=x_in[:, q4 * 256:(q4 + 1) * 256],
                                in_=x4[tcn * 4 + ti][:, q4 * 256:(q4 + 1) * 256])
                        for ds in range(KT):
                            pst = psA.tile([128, 128], F32R, tag="tp")
                            nc.tensor.transpose(
                                pst, x_in[:, ds * 128:(ds + 1) * 128], id_sb)
                            nc.vector.tensor_copy(
                                out=xT[:, ds, ti * 128:(ti + 1) * 128], in_=pst)
                    # qT / kT (channel-major)
                    for ct in range(2 * CT):
                        ps = psQ.tile([128, 512], F32, tag="qk")
                        for kt in range(KT):
                            nc.tensor.matmul(
                                ps, w_sb[:, kt, ct * 128:(ct + 1) * 128],
                                xT[:, kt, :],
                                start=(kt == 0), stop=(kt == KT - 1))
                        dst = qT if ct < CT else kT
                        nc.vector.tensor_scalar_add(
                            out=dst[:, ct % CT, tcn * 512:(tcn + 1) * 512],
                            in0=ps, scalar1=bqk_sb[:, ct:ct + 1])
                    # v (token-major, pair-packed with ones cols)
                    for ti in range(4):
                        tt = tcn * 4 + ti
                        psv = psQ.tile([128, CH], F32, tag="v")
                        for kt in range(KT):
                            nc.tensor.matmul(
                                psv, xT[:, kt, ti * 128:(ti + 1) * 128],
                                w_sb[:, kt, 2 * CH:3 * CH],
                                start=(kt == 0), stop=(kt == KT - 1))
                        psv4 = psv.rearrange("p (j two c) -> p j two c", two=2, c=64)
                        bv4 = bv_sb.rearrange("p (j two c) -> p j two c", two=2, c=64)
                        nc.vector.tensor_add(
                            out=vaug[:, tt, :, 0:64], in0=psv4[:, :, 0, :],
                            in1=bv4[:, :, 0, :])
                        nc.vector.tensor_add(
                            out=vaug[:, tt, :, 65:129], in0=psv4[:, :, 1, :],
                            in1=bv4[:, :, 1, :])

            # ---------------- Phase C: attention ------------------------------
            with tc.tile_pool(name="aout", bufs=1) as ao:
                attn_outT = ao.tile([128, CT, S], F32R)
                wo_sb = ao.tile([128, CT, D], F32R)
                bout_sb = ao.tile([128, D], F32)
                nc.sync.dma_start(out=bout_sb, in_=bout.ap())
                for dt in range(CT):
                    nc.sync.dma_start(out=wo_sb[:, dt, :], in_=wo3[dt])
                with (
                    tc.tile_pool(name="pT", bufs=3) as pTp,
                    tc.tile_pool(name="rcp", bufs=2) as rcp,
                    tc.tile_pool(name="rb", bufs=2) as rbp,
                    tc.tile_pool(name="psS", bufs=2, space="PSUM") as psS,
                    tc.tile_pool(name="psAt", bufs=4, space="PSUM") as psAt,
                ):
                    for j in range(NPAIR):
                        for qc in range(SQC):
                            q0 = qc * 1024
                            aA = [psAt.tile([65, 512], F32, tag="at", name=f"aA{j}_{qc}_{h2}") for h2 in range(2)]
                            aB = [psAt.tile([65, 512], F32, tag="at", name=f"aB{j}_{qc}_{h2}") for h2 in range(2)]
                            for kt in range(TT):
                                k0 = kt * 128
                                # One psum tile holds BOTH heads' scores for a
                                # 512-query chunk: cols 0:512 head A (rows
                                # 0:64 of the array), cols 512:1024 head B
                                # (rows 64:128). The two matmuls have no
                                # mutual deps and disjoint row groups, so the
                                # PE runs them concurrently; the exp consumes
                                # both at once.
                                for h2 in range(2):
                                    qq = q0 + h2 * 512
                                    s = psS.tile([128, 1024], F32, tag="s",
                                                 name=f"s{j}_{qc}_{kt}_{h2}")
                                    nc.tensor.matmul(
                                        s[:, 0:512],
                                        kT[0:64, j, k0:k0 + 128],
                                        qT[0:64, j, qq:qq + 512],
                                        start=True, stop=True, tile_position=(0, 0))
                                    nc.tensor.matmul(
                                        s[:, 512:1024],
                                        kT[64:128, j, k0:k0 + 128],
                                        qT[64:128, j, qq:qq + 512],
                                        start=True, stop=True, tile_position=(64, 0))
                                    pT = pTp.tile([128, 1024], F32R, tag="p",
                                                  name=f"pT{j}_{qc}_{kt}_{h2}")
                                    nc.scalar.activation(out=pT, in_=s, func=EXP)
                                    nc.tensor.matmul(
                                        aA[h2], vaug[:, kt, j, 0:65],
                                        pT[:, 0:512],
                                        start=(kt == 0), stop=(kt == TT - 1))
                                    nc.tensor.matmul(
                                        aB[h2], vaug[:, kt, j, 65:130],
                                        pT[:, 512:1024],
                                        start=(kt == 0), stop=(kt == TT - 1))
                            # evacuate + normalize.
                            # Denominators (psum row 64 of the 4 attn psums)
                            # -> DRAM -> one [128, 16] tile so the reciprocal
                            # runs 128 partitions wide (a [1, 512] DVE
                            # reciprocal costs 3.2us) -> DRAM -> partition-
                            # broadcast back as [64, 512] tiles.
                            dsb = rcp.tile([128, 512], F32, tag="ds")
                            nc.vector.tensor_copy(out=dsb[0:1, :], in_=aA[0][64:65, :])
                            nc.vector.tensor_copy(out=dsb[32:33, :], in_=aA[1][64:65, :])
                            nc.vector.tensor_copy(out=dsb[64:65, :], in_=aB[0][64:65, :])
                            nc.vector.tensor_copy(out=dsb[96:97, :], in_=aB[1][64:65, :])
                            drec = dr.tile([4, 512], F32, tag="d")
                            if j == NPAIR - 1 and qc == SQC - 1:
                                # final pair: shortest-latency path (3.2us DVE
                                # recip, one DRAM hop) since nothing overlaps
                                # the tail
                                rsb = rcp.tile([128, 512], F32, tag="ds")
                                nc.vector.reciprocal(
                                    out=rsb[0:97, :], in_=dsb[0:97, :])
                                nc.sync.dma_start(out=drec, in_=rsb[::32, :])
                            else:
                                dden = dr.tile([4, 512], F32, tag="d")
                                nc.sync.dma_start(out=dden, in_=dsb[::32, :])
                                dgat = rcp.tile([128, 16], F32, tag="g")
                                nc.sync.dma_start(
                                    out=dgat,
                                    in_=dden.rearrange("f (i p) -> p f i", p=128))
                                rgat = rcp.tile([128, 16], F32, tag="g")
                                nc.vector.reciprocal(out=rgat, in_=dgat)
                                nc.sync.dma_start(
                                    out=drec.rearrange("f (i p) -> p f i", p=128),
                                    in_=rgat)
                            # evacuate psums unnormalized (frees the attn psum
                            # banks without waiting on the reciprocal chain)
                            for h2 in range(2):
                                c0 = q0 + h2 * 512
                                nc.vector.tensor_copy(
                                    out=attn_outT[0:64, j, c0:c0 + 512],
                                    in_=aA[h2][0:64, :])
                                nc.vector.tensor_copy(
                                    out=attn_outT[64:128, j, c0:c0 + 512],
                                    in_=aB[h2][0:64, :])
                            rb4 = rbp.tile([128, 2, 512], F32, tag="rb")
                            for i in range(4):
                                p0 = 0 if i < 2 else 64
                                nc.gpsimd.dma_start(
                                    out=rb4[p0:p0 + 64, i % 2, :],
                                    in_=drec[i:i + 1, :].to_broadcast([64, 512]))
                            for h2 in range(2):
                                c0 = q0 + h2 * 512
                                nc.vector.tensor_mul(
                                    out=attn_outT[0:64, j, c0:c0 + 512],
                                    in0=attn_outT[0:64, j, c0:c0 + 512],
                                    in1=rb4[0:64, h2, :])
                                nc.vector.tensor_mul(
                                    out=attn_outT[64:128, j, c0:c0 + 512],
                                    in0=attn_outT[64:128, j, c0:c0 + 512],
                                    in1=rb4[64:128, h2, :])

                # ---------------- Phase D: output projection ------------------
                with (
                    tc.tile_pool(name="osb", bufs=3) as osb,
                    tc.tile_pool(name="psD", bufs=4, space="PSUM") as psD,
                ):
                    for tt in range(TT):
                        pd = [psD.tile([128, 512], F32, tag="pd", name=f"pd{tt}_{h2}") for h2 in range(2)]
                        for dt in range(CT):
                            for h2 in range(2):
                                nc.tensor.matmul(
                                    pd[h2],
                                    attn_outT[:, dt, tt * 128:(tt + 1) * 128],
                                    wo_sb[:, dt, h2 * 512:(h2 + 1) * 512],
                                    start=(dt == 0), stop=(dt == CT - 1))
                        ot = osb.tile([128, D], F32, tag="o")
                        for h2 in range(2):
                            nc.vector.tensor_add(
                                out=ot[:, h2 * 512:(h2 + 1) * 512], in0=pd[h2],
                                in1=bout_sb[:, h2 * 512:(h2 + 1) * 512])
                            nc.sync.dma_start(
                                out=o4[tt][:, h2 * 512:(h2 + 1) * 512],
                                in_=ot[:, h2 * 512:(h2 + 1) * 512])

    nc.finalize()
    return nc


def _get_nc():
    if "nc" not in _CACHED:
        _CACHED["nc"] = _build_nc()
    return _CACHED["nc"]


def _core_inputs(x, w_qkv, b_qkv, w_out, b_out):
    """Build the 8 per-core input dicts (host-side sharding)."""
    x = np.asarray(x, dtype=np.float32)
    w_qkv = np.asarray(w_qkv, dtype=np.float32)
    b_qkv = np.asarray(b_qkv, dtype=np.float32)
    w_out = np.asarray(w_out, dtype=np.float32)
    b_out = np.asarray(b_out, dtype=np.float32)

    scale = np.float32(HD ** -0.5)
    ident = np.eye(128, dtype=np.float32)
    bout_half = (b_out * np.float32(1.0 / TPW)).astype(np.float32)

    in_maps = []
    for core in range(NCORES):
        b, r = divmod(core, TPW)
        cs = slice(CH * r, CH * (r + 1))
        wq = w_qkv[:, 0:D][:, cs] * scale
        wk = w_qkv[:, D:2 * D][:, cs]
        wv = w_qkv[:, 2 * D:3 * D][:, cs]
        w_pack = np.ascontiguousarray(
            np.concatenate([wq, wk, wv], axis=1), dtype=np.float32)
        bqk_pack = np.concatenate(
            [b_qkv[0:D][cs] * scale, b_qkv[D:2 * D][cs]])
        # [2CH] -> [128, 2CT] with bqk[ct*128+p] at [p, ct]
        bqk_t = np.ascontiguousarray(
            bqk_pack.reshape(-1, 128).T, dtype=np.float32)
        bv_r = np.ascontiguousarray(
            np.broadcast_to(b_qkv[2 * D:3 * D][cs], (128, CH)), dtype=np.float32)
        bout_r = np.ascontiguousarray(
            np.broadcast_to(bout_half, (128, D)), dtype=np.float32)
        wout_r = np.ascontiguousarray(w_out[cs, :], dtype=np.float32)
        in_maps.append({
            "x": np.ascontiguousarray(x[b]),
            "wqkv": w_pack,
            "bqk": bqk_t,
            "bv": bv_r,
            "wout": wout_r,
            "bout": bout_r,
            "ident": ident,
        })
    return in_maps


def _ensure_ntff_hook():
    """Register the axon NTFF profile hook (missing antenv.axon_hooks stub)."""
    import sys
    import types

    if "antenv.axon_hooks" in sys.modules:
        return
    try:
        from trn_agent_boot.trn_boot import _ntff_profile_via_ctypes

        hook = _ntff_profile_via_ctypes("/opt/axon/libaxon_pjrt.so")
        if hook is None:
            return
        mod = types.ModuleType("antenv.axon_hooks")
        mod.get_axon_ntff_profile_hook = lambda: hook
        mod.set_axon_ntff_profile_hook = lambda h: None
        sys.modules["antenv.axon_hooks"] = mod
    except Exception:
        pass


def kernel(x, w_qkv, b_qkv, w_out, b_out):
    global LAST_RESULTS
    from concourse.bass_utils import run_bass_kernel_spmd

    nc = _get_nc()
    in_maps = _core_inputs(x, w_qkv, b_qkv, w_out, b_out)
    trace = bool(os.environ.get("BASS_TRACE"))
    if trace:
        _ensure_ntff_hook()
    res = run_bass_kernel_spmd(
        nc, in_maps, core_ids=list(range(NCORES)), trace=trace)
    LAST_RESULTS = res
    out = np.empty((B, S, D), dtype=np.float32)
    for b in range(B):
        out[b] = res.results[TPW * b]["o"]
        for r in range(1, TPW):
            out[b] += res.results[TPW * b + r]["o"]
    return out
